# revision 1
# baseline (speedup 1.0000x reference)
"""Mass-spring substep integrator on 8 Trainium2 NeuronCores.

Topology: all 8 cores carry the full node state for ALL 4 batch elements;
the 400k springs are split into 8 per-node-balanced shards, one per core.
Each substep every core computes the partial per-node forces of its shard
for all 4 batches at once, the partials are AllReduced across the 8 cores,
and every core integrates the full state identically.

Per-core data layout ("owner grid"):
  - nodes are relabeled on the host: sorted by incidence count and dealt
    round-robin onto the 128 SBUF partitions -> node (p, k).
  - the directed incidences (edge endpoints) of a core's shard are laid out
    in a [128, J] slot grid grouped by owner node, with a degree-profile
    template (segment sizes per rank k) SHARED across partitions and shards,
    so owner-side broadcast / segmented reduction are plain strided
    (lockstep) vector ops.
  - the partner position of every slot (all 4 batches x 3 comps = one
    48-byte record) is fetched with indirect DMA from a DRAM copy of the
    positions, one gather column (128 slots) per call.

All node/slot index tables are precomputed on the host from the (static)
edge list; outputs are un-permuted back to the original node order on the
host after the device run.
"""

import numpy as np

import concourse.bass as bass
import concourse.mybir as mybir
import concourse.tile as tile
from concourse.bass_utils import run_bass_kernel_spmd

# Problem constants (must match the reference)
B, NV, NE, SUBSTEPS = 4, 100000, 400000, 10
DT = 0.01
K_SPRING = 1000.0
MASS = 1.0
DAMP = 0.999
ACT_SCALE = 0.1
EPS = 1e-6
GRAVITY_Y = -9.8

P = 128           # SBUF partitions
NSHARD = 8        # edge shards == cores
PAD_REST = float(np.sqrt(EPS))  # rest length that zeroes force on d=0 pad slots


# ---------------------------------------------------------------------------
# walrus workaround: this toolchain accepts only ONE sync-wait per
# instruction; split extra waits onto fresh same-engine NOPs.
# ---------------------------------------------------------------------------
_ctr = [0]


def _split_multi_waits(nc):
    for f in nc.m.functions:
        for b in f.blocks:
            old = b.instructions
            new = []
            changed = False
            for inst in old:
                si = inst.sync_info
                if si is not None and si.on_wait is not None and len(si.on_wait) > 1:
                    waits = list(si.on_wait)
                    for w in waits[:-1]:
                        _ctr[0] += 1
                        nop = mybir.InstNoOp(
                            name=f"SPLITW-{_ctr[0]}",
                            engine=inst.engine,
                            ins=[], outs=[],
                            sync_info=mybir.SyncInfo(on_wait=[w], on_update=[]),
                        )
                        new.append(nop)
                    si.on_wait = waits[-1:]
                    changed = True
                new.append(inst)
            if changed:
                b.instructions = new


class _TileContext(tile.TileContext):
    def __exit__(self, *args):
        r = super().__exit__(*args)
        if args[0] is None:
            _split_multi_waits(self.nc)
        return r


# ---------------------------------------------------------------------------
# Host-side plan construction (static, depends only on the edge list)
# ---------------------------------------------------------------------------
class Plan:
    pass


def build_plan(edges, nv, ne):
    """Relabel nodes, split edges into NSHARD balanced shards, build the
    shared degree-profile slot template and per-shard index tables."""
    rng = np.random.RandomState(0)
    nvp = -(-nv // P)            # nodes per partition (ceil)
    nvtot = nvp * P

    i_idx = edges[:, 0].astype(np.int64)
    j_idx = edges[:, 1].astype(np.int64)

    # --- balanced split of edges into NSHARD shards (per-node incidence) ---
    order = rng.permutation(ne)
    cnt = np.zeros((NSHARD, nv), np.int32)
    shard_of_edge = np.zeros(ne, np.int8)
    ii, jj = i_idx[order], j_idx[order]
    for t in range(ne):
        u = ii[t]
        v = jj[t]
        s = int(np.argmin(cnt[:, u] + cnt[:, v]))
        shard_of_edge[order[t]] = s
        cnt[s, u] += 1
        cnt[s, v] += 1

    deg_h = cnt  # [NSHARD, NV]

    # --- node ranking: sort by max shard-degree desc, deal round-robin ---
    key = deg_h.max(axis=0)
    node_order = np.argsort(-key, kind="stable")
    node_order_pad = np.concatenate([node_order, np.arange(nv, nvtot)])
    grid_nodes = node_order_pad.reshape(nvp, P)  # [k, p]
    p_of = np.zeros(nvtot, np.int32)
    k_of = np.zeros(nvtot, np.int32)
    p_of[grid_nodes.ravel()] = np.tile(np.arange(P), nvp)
    k_of[grid_nodes.ravel()] = np.repeat(np.arange(nvp), P)

    # --- shared degree template: D[k] = max over shards & partitions ---
    degh_pad = np.zeros((NSHARD, nvtot), np.int32)
    degh_pad[:, :nv] = deg_h
    dk = np.max(degh_pad[:, grid_nodes], axis=(0, 2))  # [nvp]
    rk_order = np.argsort(-dk, kind="stable")
    grid_nodes = grid_nodes[rk_order]
    dk = dk[rk_order]
    k_of[grid_nodes.ravel()] = np.repeat(np.arange(nvp), P)

    seg_start = np.zeros(nvp + 1, np.int64)
    seg_start[1:] = np.cumsum(dk)
    J = int(seg_start[-1])

    # degree classes: runs of equal dk with dk >= 1
    classes = []
    k = 0
    while k < nvp:
        d = int(dk[k])
        k2 = k
        while k2 < nvp and dk[k2] == d:
            k2 += 1
        if d >= 1:
            classes.append((k, k2, d))
        k = k2

    # split classes into chunks of bounded slot count (for SBUF);
    # classes may split at rank boundaries.
    nchunk = 3
    target = -(-J // nchunk)
    cls_chunks = [[]]
    cur = 0
    for (ka, kb, d) in classes:
        k0 = ka
        while k0 < kb:
            room = max((target - cur) // d, 0)
            take = min(kb - k0, room)
            if take == 0:
                cls_chunks.append([])
                cur = 0
                continue
            cls_chunks[-1].append((k0, k0 + take, d))
            cur += take * d
            k0 += take
    cls_chunks = [ch for ch in cls_chunks if ch]
    chunk_bounds = [
        (int(seg_start[ch[0][0]]), int(seg_start[ch[-1][1]]))
        for ch in cls_chunks
    ]

    # --- per-shard slot tables ---
    flat_of = (p_of.astype(np.int64) * nvp + k_of)
    part_idx = np.zeros((NSHARD, P, J), np.int32)
    eidx_slot = np.full((NSHARD, P, J), -1, np.int64)

    owner_flat = np.zeros((P, J), np.int64)
    for (ka, kb, d) in classes:
        for krank in range(ka, kb):
            s0 = seg_start[krank]
            owner_flat[:, s0:s0 + d] = (
                np.arange(P, dtype=np.int64)[:, None] * nvp + krank
            )

    for h in range(NSHARD):
        sel = shard_of_edge == h
        eu = np.concatenate([i_idx[sel], j_idx[sel]])
        ev = np.concatenate([j_idx[sel], i_idx[sel]])
        ee = np.concatenate([np.nonzero(sel)[0]] * 2)
        owner_p = p_of[eu]
        owner_k = k_of[eu]
        so = np.lexsort((ee, owner_k, owner_p))
        eu, ev, ee = eu[so], ev[so], ee[so]
        owner_p, owner_k = owner_p[so], owner_k[so]
        grp = owner_p.astype(np.int64) * nvp + owner_k
        uniq, first = np.unique(grp, return_index=True)
        within = np.arange(len(grp)) - np.repeat(
            first, np.diff(np.append(first, len(grp))))
        jpos = seg_start[owner_k] + within
        part_idx[h, owner_p, jpos] = flat_of[ev]
        eidx_slot[h, owner_p, jpos] = ee
        padmask = eidx_slot[h] < 0
        part_idx[h][padmask] = owner_flat[padmask].astype(np.int32)

    plan = Plan()
    plan.nv, plan.ne, plan.nvp, plan.nvtot, plan.J = nv, ne, nvp, nvtot, J
    plan.classes = classes
    plan.cls_chunks = cls_chunks
    plan.chunk_bounds = chunk_bounds
    plan.seg_start = seg_start
    plan.part_idx = part_idx
    plan.eidx_slot = eidx_slot
    plan.p_of, plan.k_of = p_of, k_of
    plan.grid_nodes = grid_nodes
    return plan


def host_state_inputs(plan, input_pos, input_vel):
    """Shared (all-core) initial state in internal layout [P, M*nvp],
    plane m = b*3 + c."""
    nvp = plan.nvp
    nv = plan.nv
    nb = input_pos.shape[0]
    gn = plan.grid_nodes  # [k, p]
    valid = gn < nv
    gp = np.clip(gn, 0, nv - 1)
    ps = input_pos[:, gp].copy()   # [b, k, p, 3]
    vs = input_vel[:, gp].copy()
    ps[:, ~valid] = 0.0
    vs[:, ~valid] = 0.0
    pos = ps.transpose(2, 0, 3, 1).reshape(P, nb * 3 * nvp)
    vel = vs.transpose(2, 0, 3, 1).reshape(P, nb * 3 * nvp)
    return (np.ascontiguousarray(pos, dtype=np.float32),
            np.ascontiguousarray(vel, dtype=np.float32))


def host_shard_inputs(plan, h, input_action, rest_len):
    """Per-core shard tables: pidx [P,J] i32, rest [P,J] f32,
    act [P, J*NB] f32 (b innermost)."""
    J = plan.J
    nb = input_action.shape[0]
    e = plan.eidx_slot[h]
    pad = e < 0
    ec = np.clip(e, 0, plan.ne - 1)
    rest = rest_len[ec].astype(np.float32)
    rest[pad] = PAD_REST
    act = input_action[:, ec].astype(np.float32)  # [b, P, J]
    act[:, pad] = 0.0
    act = np.ascontiguousarray(act.transpose(1, 2, 0).reshape(P, J * nb))
    return {
        "pidx": np.ascontiguousarray(plan.part_idx[h]),
        "rest_s": np.ascontiguousarray(rest),
        "act_s": act,
    }


def unpermute_output(plan, traj, nb):
    """traj [S+1, P, nb*3, nvp] internal -> [nb, S+1, NV, 3]."""
    pv = plan.p_of[: plan.nv]
    kv = plan.k_of[: plan.nv]
    t = traj.reshape(traj.shape[0], P, nb, 3, plan.nvp)
    out = t[:, pv, :, :, kv]        # [NV, S+1, nb, 3]
    return np.ascontiguousarray(out.transpose(2, 1, 0, 3))


# ---------------------------------------------------------------------------
# Device kernel
# ---------------------------------------------------------------------------
def build_bass(plan, substeps, nb):
    nvp, J, nvtot = plan.nvp, plan.J, plan.nvtot
    m = nb * 3
    NPM = m * nvp
    f32 = mybir.dt.float32

    nc = bass.Bass(num_devices=8)
    pos0 = nc.dram_tensor("pos0", [P, NPM], f32, kind="ExternalInput")
    vel0 = nc.dram_tensor("vel0", [P, NPM], f32, kind="ExternalInput")
    pidx = nc.dram_tensor("pidx", [P, J], mybir.dt.int32, kind="ExternalInput")
    rest_in = nc.dram_tensor("rest_s", [P, J], f32, kind="ExternalInput")
    act_in = nc.dram_tensor("act_s", [P, J * nb], f32, kind="ExternalInput")

    opos = nc.dram_tensor("opos", [substeps + 1, P, NPM], f32,
                          kind="ExternalOutput")
    ovel = nc.dram_tensor("ovel", [substeps + 1, P, NPM], f32,
                          kind="ExternalOutput")

    ptab = nc.dram_tensor("ptab", [nvtot, m], f32, kind="Internal")
    cc_in = nc.dram_tensor("cc_in", [P, NPM], f32, kind="Internal")
    cc_out = nc.dram_tensor("cc_out", [P, NPM], f32, kind="Internal")

    chunks = plan.chunk_bounds
    maxchunk = max(hi - lo for (lo, hi) in chunks)

    with _TileContext(nc) as tc:
        with tc.tile_pool(name="state", bufs=1) as pool:
            pos = pool.tile([P, NPM], f32, name="pos")
            vel = pool.tile([P, NPM], f32, name="vel")
            fsum = pool.tile([P, NPM], f32, name="fsum")
            pidx_sb = pool.tile([P, J], mybir.dt.int32, name="pidx_sb")
            kr = pool.tile([P, J * nb], f32, name="kr")
            s2 = pool.tile([P, J * nb], f32, name="s2")
            sq = pool.tile([P, maxchunk * nb], f32, name="sq")
            rem = pool.tile([P, maxchunk * m], f32, name="rem")
            eps_t = pool.tile([P, 1], f32, name="eps_t")

            pos_mk = pos[:].rearrange("p (m k) -> p m k", m=m)
            fsum_mk = fsum[:].rearrange("p (m k) -> p m k", m=m)

            def _ins_bcast(ap, pos_idx, count):
                dims = [list(x) for x in ap.ap]
                dims.insert(pos_idx, [0, count])
                return bass.AP(ap.tensor, ap.offset, dims)

            # ---- one-time setup ----
            nc.vector.memset(eps_t[:], float(EPS))
            nc.sync.dma_start(pos[:], pos0[:])
            nc.sync.dma_start(vel[:], vel0[:])
            nc.sync.dma_start(pidx_sb[:], pidx[:])
            # kr[p, j, b] = K * rest[p, j] * (1 + ACT_SCALE * tanh(act))
            act_t = s2[:]
            nc.sync.dma_start(act_t, act_in[:])
            nc.scalar.activation(kr[:], act_t,
                                 mybir.ActivationFunctionType.Tanh)
            nc.vector.tensor_scalar(
                out=kr[:], in0=kr[:], scalar1=float(ACT_SCALE),
                scalar2=float(1.0), op0=mybir.AluOpType.mult,
                op1=mybir.AluOpType.add)
            rest_t = rem[:, 0:J]
            nc.sync.dma_start(rest_t, rest_in[:])
            kr_v = kr[:].rearrange("p (j b) -> p j b", b=nb)
            rest_b = _ins_bcast(rest_t, 2, nb)
            nc.vector.tensor_tensor(out=kr_v, in0=kr_v, in1=rest_b,
                                    op=mybir.AluOpType.mult)
            nc.vector.tensor_scalar_mul(kr[:], kr[:], float(K_SPRING))

            # initial state into trajectory
            nc.sync.dma_start(opos[0], pos[:])
            nc.sync.dma_start(ovel[0], vel[:])

            # ---- substeps (statically unrolled) ----
            for s in range(substeps):
                # 1) node positions -> DRAM table [nvtot, m]
                for mm in range(m):
                    for ph in (0, 1):
                        pr = ptab[ph * 64 * nvp:(ph + 1) * 64 * nvp,
                                  mm:mm + 1]
                        nc.sync.dma_start(
                            pr.rearrange("(p k) o -> p k o", p=64),
                            pos[ph * 64:(ph + 1) * 64,
                                mm * nvp:(mm + 1) * nvp],
                        )
                nc.vector.memset(fsum[:], 0.0)

                for ci, (lo, hi) in enumerate(chunks):
                    cw = hi - lo
                    rem_v = rem[:, :cw * m].rearrange(
                        "p (j r) -> p j r", r=m)          # [P, cw, m]
                    # 2) bridge: one gather column per slot
                    for j in range(lo, hi):
                        nc.gpsimd.indirect_dma_start(
                            out=rem[:, (j - lo) * m:(j - lo + 1) * m],
                            out_offset=None,
                            in_=ptab[:],
                            in_offset=bass.IndirectOffsetOnAxis(
                                ap=pidx_sb[:, j:j + 1], axis=0),
                        )
                    # 3) d = rem - own (per degree class)
                    for (ka, kb, d) in plan.cls_chunks[ci]:
                        s0 = int(plan.seg_start[ka]) - lo
                        nk = kb - ka
                        dst = rem_v[:, s0:s0 + nk * d, :].rearrange(
                            "p (n dd) r -> p n dd r", dd=d)
                        src = pos_mk[:, :, ka:kb].rearrange("p m n -> p n m")
                        src = _ins_bcast(src, 2, d)
                        nc.vector.tensor_tensor(
                            out=dst, in0=dst, in1=src,
                            op=mybir.AluOpType.subtract)
                    # 4) s2[j, b] = sum_c d_c^2
                    s2c = s2[:, lo * nb:hi * nb]
                    s2v = s2c.rearrange("p (j b) -> p j b", b=nb)
                    sqc = sq[:, :cw * nb]
                    sqv = sqc.rearrange("p (j b) -> p j b", b=nb)
                    rem_jbc = rem[:, :cw * m].rearrange(
                        "p (j b c) -> p j b c", b=nb, c=3)
                    cviews = [rem_jbc[:, :, :, c] for c in range(3)]
                    nc.vector.tensor_tensor(out=s2v, in0=cviews[0],
                                            in1=cviews[0],
                                            op=mybir.AluOpType.mult)
                    for c in (1, 2):
                        nc.vector.tensor_tensor(out=sqv, in0=cviews[c],
                                                in1=cviews[c],
                                                op=mybir.AluOpType.mult)
                        nc.vector.tensor_tensor(out=s2v, in0=s2v, in1=sqv,
                                                op=mybir.AluOpType.add)
                    # length = sqrt(s2+eps); inv = 1/length (into sq)
                    nc.scalar.activation(
                        s2c, s2c, mybir.ActivationFunctionType.Sqrt,
                        bias=eps_t[:])
                    nc.vector.reciprocal(sqc, s2c)
                    # coef = K - kr/len   (into s2)
                    nc.vector.tensor_tensor(
                        out=s2c, in0=sqc, in1=kr[:, lo * nb:hi * nb],
                        op=mybir.AluOpType.mult)
                    nc.scalar.activation(
                        s2c, s2c, mybir.ActivationFunctionType.Copy,
                        bias=float(K_SPRING), scale=-1.0)
                    # 5) f = coef * d (in place)
                    coef_b = _ins_bcast(s2v, 3, 3)
                    nc.vector.tensor_tensor(
                        out=rem_jbc, in0=rem_jbc, in1=coef_b,
                        op=mybir.AluOpType.mult)
                    # 6) segmented reduce -> fsum planes
                    for (ka, kb, d) in plan.cls_chunks[ci]:
                        s0 = int(plan.seg_start[ka]) - lo
                        nk = kb - ka
                        src = rem_v[:, s0:s0 + nk * d, :].rearrange(
                            "p (n dd) r -> p n r dd", dd=d)
                        dst = fsum_mk[:, :, ka:kb].rearrange("p m n -> p n m")
                        nc.vector.tensor_reduce(
                            out=dst, in_=src, axis=mybir.AxisListType.X,
                            op=mybir.AluOpType.add)

                # 7) AllReduce partial forces across the 8 shards
                nc.sync.dma_start(cc_in[:], fsum[:])
                nc.gpsimd.collective_compute(
                    "AllReduce", mybir.AluOpType.add,
                    replica_groups=[list(range(8))],
                    ins=[cc_in[:]], outs=[cc_out[:]],
                )
                nc.sync.dma_start(fsum[:], cc_out[:])
                # 8) integrate:
                #    fsum = fsum*DT + vel ; fsum_y += DT*G (per batch)
                #    vel = fsum*DAMP ; pos = vel*DT + pos
                nc.vector.scalar_tensor_tensor(
                    out=fsum[:], in0=fsum[:], scalar=float(DT / MASS),
                    in1=vel[:], op0=mybir.AluOpType.mult,
                    op1=mybir.AluOpType.add)
                for b in range(nb):
                    mm = b * 3 + 1
                    nc.vector.tensor_scalar_add(
                        fsum[:, mm * nvp:(mm + 1) * nvp],
                        fsum[:, mm * nvp:(mm + 1) * nvp],
                        float(GRAVITY_Y * DT))
                nc.vector.tensor_scalar_mul(vel[:], fsum[:], float(DAMP))
                nc.vector.scalar_tensor_tensor(
                    out=pos[:], in0=vel[:], scalar=float(DT),
                    in1=pos[:], op0=mybir.AluOpType.mult,
                    op1=mybir.AluOpType.add)
                # 9) write trajectory
                nc.sync.dma_start(opos[s + 1], pos[:])
                nc.sync.dma_start(ovel[s + 1], vel[:])

    return nc


# ---------------------------------------------------------------------------
# Entry point
# ---------------------------------------------------------------------------
_cache = {}


def _get_plan_and_bass(edges, nv, ne, substeps, nb):
    kh = (hash(edges.tobytes()), nv, ne, substeps, nb)
    if kh not in _cache:
        plan = build_plan(edges, nv, ne)
        nc = build_bass(plan, substeps, nb)
        _cache[kh] = (plan, nc)
    return _cache[kh]


def kernel(input_action, input_pos, input_vel, rest_len, edges):
    input_action = np.asarray(input_action, np.float32)
    input_pos = np.asarray(input_pos, np.float32)
    input_vel = np.asarray(input_vel, np.float32)
    rest_len = np.asarray(rest_len, np.float32)
    edges = np.asarray(edges, np.int32)

    nb, nv, _ = input_pos.shape
    ne = edges.shape[0]
    plan, nc = _get_plan_and_bass(edges, nv, ne, SUBSTEPS, nb)

    pos0, vel0 = host_state_inputs(plan, input_pos, input_vel)
    in_maps = []
    for c in range(8):
        im = {"pos0": pos0, "vel0": vel0}
        im.update(host_shard_inputs(plan, c, input_action, rest_len))
        in_maps.append(im)
    res = run_bass_kernel_spmd(nc, in_maps, core_ids=list(range(8)))

    r = res.results[0]
    tp = r["opos"].reshape(SUBSTEPS + 1, P, nb * 3, plan.nvp)
    tv = r["ovel"].reshape(SUBSTEPS + 1, P, nb * 3, plan.nvp)
    out_pos = unpermute_output(plan, tp, nb)
    out_vel = unpermute_output(plan, tv, nb)
    return out_pos, out_vel



# revision 6
# speedup vs baseline: 3.4388x; 3.4388x over previous
"""Mass-spring substep integrator on 8 Trainium2 NeuronCores.

Topology ("solo" data-parallel): the 4 batches are fully independent
simulations, so core c simulates batch c % 4 end-to-end (batches are
duplicated onto cores 4-7; the host reads cores 0-3). No collectives.

Per-core data layout:
  - nodes are relabeled on the host: sorted by incidence count and dealt
    in groups of 128 onto the SBUF partitions -> node (p, k); state is
    node-major per partition: pos[p, k*3 + c]. The DRAM gather table
    ptab[(p*nvp + k), c] is then a single contiguous DMA away from the
    SBUF state.
  - the directed incidences (edge endpoints) are laid out in a [128, J]
    slot grid grouped by owner node, with a degree-profile template
    (segment size per rank k) shared across partitions, so owner-side
    broadcast / segmented reduction are plain strided vector ops.
  - partner positions for a whole chunk of slot columns are fetched with
    ONE batched indirect DMA per chunk (offset AP [128, cw]) from ptab.

All index tables are precomputed on the host from the (static) edge
list; outputs are un-permuted back to the original node order on the
host after the device run.
"""

import numpy as np

import concourse.bass as bass
import concourse.mybir as mybir
import concourse.tile as tile
from concourse.bass_utils import run_bass_kernel_spmd

# Problem constants (must match the reference)
B, NV, NE, SUBSTEPS = 4, 100000, 400000, 10
DT = 0.01
K_SPRING = 1000.0
MASS = 1.0
DAMP = 0.999
ACT_SCALE = 0.1
EPS = 1e-6
GRAVITY_Y = -9.8

P = 128           # SBUF partitions
NCHUNK = 5        # slot-column chunks per substep
PAD_REST = float(np.sqrt(EPS))  # rest length that zeroes force on d=0 pad slots


# ---------------------------------------------------------------------------
# walrus workaround: this toolchain accepts only ONE sync-wait per
# instruction; split extra waits onto fresh same-engine NOPs.
# ---------------------------------------------------------------------------
_ctr = [0]


def _split_multi_waits(nc):
    for f in nc.m.functions:
        for b in f.blocks:
            old = b.instructions
            new = []
            changed = False
            for inst in old:
                si = inst.sync_info
                if si is not None and si.on_wait is not None and len(si.on_wait) > 1:
                    waits = list(si.on_wait)
                    for w in waits[:-1]:
                        _ctr[0] += 1
                        nop = mybir.InstNoOp(
                            name=f"SPLITW-{_ctr[0]}",
                            engine=inst.engine,
                            ins=[], outs=[],
                            sync_info=mybir.SyncInfo(on_wait=[w], on_update=[]),
                        )
                        new.append(nop)
                    si.on_wait = waits[-1:]
                    changed = True
                new.append(inst)
            if changed:
                b.instructions = new


class _TileContext(tile.TileContext):
    def __exit__(self, *args):
        r = super().__exit__(*args)
        if args[0] is None:
            _split_multi_waits(self.nc)
        return r


# ---------------------------------------------------------------------------
# Host-side plan construction (static, depends only on the edge list)
# ---------------------------------------------------------------------------
class Plan:
    pass


def build_plan(edges, nv, ne):
    """Relabel nodes by degree, build the shared degree-profile slot
    template and the slot index tables (identical for every core since
    all batches share the topology)."""
    nvp = -(-nv // P)            # nodes per partition (ceil)
    nvtot = nvp * P

    i_idx = edges[:, 0].astype(np.int64)
    j_idx = edges[:, 1].astype(np.int64)

    deg = np.bincount(edges.ravel(), minlength=nv).astype(np.int64)
    deg_pad = np.concatenate([deg, np.zeros(nvtot - nv, np.int64)])

    # rank k <- the k-th group of 128 nodes in degree-sorted order
    order = np.argsort(-deg_pad, kind="stable")
    grid_nodes = order.reshape(nvp, P)          # [k, p]
    p_of = np.zeros(nvtot, np.int32)
    k_of = np.zeros(nvtot, np.int32)
    p_of[grid_nodes.ravel()] = np.tile(np.arange(P, dtype=np.int32), nvp)
    k_of[grid_nodes.ravel()] = np.repeat(np.arange(nvp, dtype=np.int32), P)

    dk = deg_pad[grid_nodes[:, 0]]              # max degree per rank group
    seg_start = np.zeros(nvp + 1, np.int64)
    seg_start[1:] = np.cumsum(dk)
    J = int(seg_start[-1])

    # degree classes: runs of equal dk with dk >= 1
    classes = []
    k = 0
    while k < nvp:
        d = int(dk[k])
        k2 = k
        while k2 < nvp and dk[k2] == d:
            k2 += 1
        if d >= 1:
            classes.append((k, k2, d))
        k = k2

    # chunks: split the slot range into NCHUNK ~equal pieces at rank
    # boundaries; record the class pieces inside each chunk.
    targets = [J * (i + 1) // NCHUNK for i in range(NCHUNK)]
    bounds_k = [0]
    for t in targets[:-1]:
        kb = int(np.searchsorted(seg_start, t, side="left"))
        kb = max(min(kb, nvp), bounds_k[-1])
        bounds_k.append(kb)
    bounds_k.append(nvp)
    chunks = []
    for ci in range(NCHUNK):
        klo, khi = bounds_k[ci], bounds_k[ci + 1]
        lo, hi = int(seg_start[klo]), int(seg_start[khi])
        if hi == lo:
            continue
        pieces = []
        for (ka, kb, d) in classes:
            a, b2 = max(ka, klo), min(kb, khi)
            if a < b2:
                pieces.append((a, b2, d))
        chunks.append(dict(klo=klo, khi=khi, lo=lo, hi=hi, pieces=pieces))

    flat_of = p_of.astype(np.int64) * nvp + k_of

    # slot fill: directed incidences grouped by owner (p, k)
    iu = np.concatenate([i_idx, j_idx])
    iv = np.concatenate([j_idx, i_idx])
    ee = np.concatenate([np.arange(ne, dtype=np.int64)] * 2)
    op_ = p_of[iu].astype(np.int64)
    ok = k_of[iu].astype(np.int64)
    so = np.lexsort((ee, ok, op_))
    op_, ok, iv, ee = op_[so], ok[so], iv[so], ee[so]
    grp = op_ * nvp + ok
    uniq, first = np.unique(grp, return_index=True)
    within = np.arange(len(grp)) - np.repeat(
        first, np.diff(np.append(first, len(grp))))
    jpos = seg_start[ok] + within

    rank_of_slot = np.repeat(np.arange(nvp, dtype=np.int64), dk)  # [J]
    own_flat = (np.arange(P, dtype=np.int64)[:, None] * nvp
                + rank_of_slot[None, :])                          # [P, J]

    pidx = own_flat.astype(np.int32).copy()
    eidx = np.full((P, J), -1, np.int64)
    pidx[op_, jpos] = flat_of[iv].astype(np.int32)
    eidx[op_, jpos] = ee

    plan = Plan()
    plan.nv, plan.ne, plan.nvp, plan.nvtot, plan.J = nv, ne, nvp, nvtot, J
    plan.classes = classes
    plan.chunks = chunks
    plan.seg_start = seg_start
    plan.pidx = np.ascontiguousarray(pidx)
    plan.eidx = eidx
    plan.p_of, plan.k_of = p_of, k_of
    plan.grid_nodes = grid_nodes
    plan.cw_max = max(c["hi"] - c["lo"] for c in chunks)
    return plan


def host_state_inputs(plan, pos_b, vel_b):
    """One batch's initial state in internal layout [P, nvp*3]."""
    nvp, nv = plan.nvp, plan.nv
    gn = plan.grid_nodes                      # [k, p]
    valid = gn < nv
    gp = np.clip(gn, 0, nv - 1)
    ps = pos_b[gp].copy()                     # [k, p, 3]
    vs = vel_b[gp].copy()
    ps[~valid] = 0.0
    vs[~valid] = 0.0
    pos = ps.transpose(1, 0, 2).reshape(P, nvp * 3)
    vel = vs.transpose(1, 0, 2).reshape(P, nvp * 3)
    return (np.ascontiguousarray(pos, dtype=np.float32),
            np.ascontiguousarray(vel, dtype=np.float32))


def host_nkr(plan, act_b, rest_len):
    """nkr[p, j] = -K * rest_eff for the slot's edge; pads get -K*PAD_REST."""
    e = plan.eidx
    pad = e < 0
    ec = np.clip(e, 0, plan.ne - 1)
    rest_eff = rest_len[ec] * (1.0 + ACT_SCALE * np.tanh(act_b[ec]))
    nkr = (-K_SPRING * rest_eff).astype(np.float32)
    nkr[pad] = -K_SPRING * PAD_REST
    return np.ascontiguousarray(nkr)


def unpermute_output(plan, traj):
    """traj [S+1, P, nvp*3] internal -> [S+1, NV, 3]."""
    s1 = traj.shape[0]
    t = traj.reshape(s1, P, plan.nvp, 3)
    pv = plan.p_of[: plan.nv]
    kv = plan.k_of[: plan.nv]
    return t[:, pv, kv, :]                    # [S+1, NV, 3]


# ---------------------------------------------------------------------------
# Device kernel
# ---------------------------------------------------------------------------
def _ins_bcast(ap, pos_idx, count):
    dims = [list(x) for x in ap.ap]
    dims.insert(pos_idx, [0, count])
    return bass.AP(ap.tensor, ap.offset, dims)


def build_bass(plan, substeps):
    nvp, J, nvtot = plan.nvp, plan.J, plan.nvtot
    npm = nvp * 3
    cwm = plan.cw_max
    f32 = mybir.dt.float32

    nc = bass.Bass(num_devices=8)
    pos0 = nc.dram_tensor("pos0", [P, npm], f32, kind="ExternalInput")
    vel0 = nc.dram_tensor("vel0", [P, npm], f32, kind="ExternalInput")
    pidx = nc.dram_tensor("pidx", [P, J], mybir.dt.int32, kind="ExternalInput")
    nkr_in = nc.dram_tensor("nkr", [P, J], f32, kind="ExternalInput")

    opos = nc.dram_tensor("opos", [substeps + 1, P, npm], f32,
                          kind="ExternalOutput")
    ovel = nc.dram_tensor("ovel", [substeps + 1, P, npm], f32,
                          kind="ExternalOutput")

    ptab = nc.dram_tensor("ptab", [nvtot, 3], f32, kind="Internal")
    ptab_pview = ptab[:].rearrange("(p k) c -> p (k c)", p=P)

    with _TileContext(nc) as tc:
        with tc.tile_pool(name="state", bufs=1) as pool:
            pos = pool.tile([P, npm], f32, name="pos")
            vel = pool.tile([P, npm], f32, name="vel")
            fsum = pool.tile([P, npm], f32, name="fsum")
            pidx_sb = pool.tile([P, J], mybir.dt.int32, name="pidx_sb")
            nkr_sb = pool.tile([P, J], f32, name="nkr_sb")
            eps_t = pool.tile([P, 1], f32, name="eps_t")
            grav_t = pool.tile([P, 1], f32, name="grav_t")
            rem = [pool.tile([P, cwm * 3], f32, name=f"rem{b}")
                   for b in range(2)]
            sq = [pool.tile([P, cwm * 3], f32, name=f"sq{b}")
                  for b in range(2)]
            s2 = [pool.tile([P, cwm], f32, name=f"s2{b}") for b in range(2)]
            inv = [pool.tile([P, cwm], f32, name=f"inv{b}") for b in range(2)]

            pos_kc = pos[:].rearrange("p (k c) -> p k c", c=3)
            fsum_kc = fsum[:].rearrange("p (k c) -> p k c", c=3)

            # ---- one-time setup ----
            nc.vector.memset(eps_t[:], float(EPS))
            nc.vector.memset(grav_t[:], float(GRAVITY_Y * DT))
            nc.vector.memset(fsum[:], 0.0)
            nc.sync.dma_start(pos[:], pos0[:])
            nc.sync.dma_start(vel[:], vel0[:])
            nc.sync.dma_start(pidx_sb[:], pidx[:])
            nc.sync.dma_start(nkr_sb[:], nkr_in[:])
            nc.sync.dma_start(opos[0], pos[:])
            nc.sync.dma_start(ovel[0], vel[:])

            # ---- substeps (statically unrolled) ----
            for s in range(substeps):
                # 1) node positions -> DRAM gather table (one contiguous DMA)
                nc.sync.dma_start(ptab_pview, pos[:])

                for ci, ch in enumerate(plan.chunks):
                    bi = ci % 2
                    lo, hi = ch["lo"], ch["hi"]
                    cw = hi - lo
                    remc = rem[bi][:, :cw * 3]
                    sqc = sq[bi][:, :cw * 3]
                    s2c = s2[bi][:, :cw]
                    invc = inv[bi][:, :cw]
                    rem_jc = remc.rearrange("p (j c) -> p j c", c=3)

                    # 2) gather partner positions (one indirect DMA)
                    nc.gpsimd.indirect_dma_start(
                        out=remc,
                        out_offset=None,
                        in_=ptab[:],
                        in_offset=bass.IndirectOffsetOnAxis(
                            ap=pidx_sb[:, lo:hi], axis=0),
                    )
                    # 3) d = partner - owner (per degree-class piece)
                    for (ka, kb, d) in ch["pieces"]:
                        s0 = int(plan.seg_start[ka]) - lo
                        nk = kb - ka
                        dst = rem_jc[:, s0:s0 + nk * d, :].rearrange(
                            "p (n dd) c -> p n dd c", dd=d)
                        src = _ins_bcast(pos_kc[:, ka:kb, :], 2, d)
                        nc.vector.tensor_tensor(
                            out=dst, in0=dst, in1=src,
                            op=mybir.AluOpType.subtract)
                    # 4) sq = d*d (ACT), s2 = sum_c sq (DVE)
                    nc.scalar.activation(
                        sqc, remc, mybir.ActivationFunctionType.Square)
                    nc.vector.tensor_reduce(
                        out=s2c, in_=sqc.rearrange("p (j c) -> p j c", c=3),
                        axis=mybir.AxisListType.X, op=mybir.AluOpType.add)
                    # 5) len = sqrt(s2 + eps) (ACT, in place)
                    nc.scalar.activation(
                        s2c, s2c, mybir.ActivationFunctionType.Sqrt,
                        bias=eps_t[:])
                    # 6) inv = 1/len ; t = nkr*inv  (t in inv, in place)
                    nc.vector.reciprocal(invc, s2c)
                    nc.vector.tensor_tensor(
                        out=invc, in0=nkr_sb[:, lo:hi], in1=invc,
                        op=mybir.AluOpType.mult)
                    # 7) f = (t + K) * d   (in place on rem)
                    tb = _ins_bcast(invc, 2, 3)
                    nc.vector.scalar_tensor_tensor(
                        out=rem_jc, in0=tb, scalar=float(K_SPRING),
                        in1=rem_jc, op0=mybir.AluOpType.add,
                        op1=mybir.AluOpType.mult)
                    # 8) segmented reduce -> fsum
                    for (ka, kb, d) in ch["pieces"]:
                        s0 = int(plan.seg_start[ka]) - lo
                        nk = kb - ka
                        src = rem_jc[:, s0:s0 + nk * d, :].rearrange(
                            "p (n dd) c -> p n c dd", dd=d)
                        dst = fsum_kc[:, ka:kb, :]
                        nc.vector.tensor_reduce(
                            out=dst, in_=src, axis=mybir.AxisListType.X,
                            op=mybir.AluOpType.add)

                # 9) integrate:
                #    t = fsum*DT + vel ; t_y += DT*G ; vel = DAMP*t ;
                #    pos = vel*DT + pos
                nc.vector.scalar_tensor_tensor(
                    out=fsum[:], in0=fsum[:], scalar=float(DT / MASS),
                    in1=vel[:], op0=mybir.AluOpType.mult,
                    op1=mybir.AluOpType.add)
                yv = fsum_kc[:, :, 1:2]
                nc.scalar.activation(
                    yv, yv, mybir.ActivationFunctionType.Identity,
                    bias=grav_t[:])
                nc.scalar.activation(
                    vel[:], fsum[:], mybir.ActivationFunctionType.Copy,
                    scale=float(DAMP))
                nc.vector.scalar_tensor_tensor(
                    out=pos[:], in0=vel[:], scalar=float(DT),
                    in1=pos[:], op0=mybir.AluOpType.mult,
                    op1=mybir.AluOpType.add)
                # 10) trajectory
                nc.sync.dma_start(opos[s + 1], pos[:])
                nc.sync.dma_start(ovel[s + 1], vel[:])

    return nc


# ---------------------------------------------------------------------------
# Entry point
# ---------------------------------------------------------------------------
_cache = {}


def _get_plan_and_bass(edges, nv, ne, substeps):
    kh = (hash(edges.tobytes()), nv, ne, substeps)
    if kh not in _cache:
        plan = build_plan(edges, nv, ne)
        nc = build_bass(plan, substeps)
        _cache[kh] = (plan, nc)
    return _cache[kh]


def kernel(input_action, input_pos, input_vel, rest_len, edges):
    input_action = np.asarray(input_action, np.float32)
    input_pos = np.asarray(input_pos, np.float32)
    input_vel = np.asarray(input_vel, np.float32)
    rest_len = np.asarray(rest_len, np.float32)
    edges = np.asarray(edges, np.int32)

    nb, nv, _ = input_pos.shape
    ne = edges.shape[0]
    plan, nc = _get_plan_and_bass(edges, nv, ne, SUBSTEPS)

    in_maps = []
    for c in range(8):
        b = c % nb
        pos0, vel0 = host_state_inputs(plan, input_pos[b], input_vel[b])
        in_maps.append({
            "pos0": pos0,
            "vel0": vel0,
            "pidx": plan.pidx,
            "nkr": host_nkr(plan, input_action[b], rest_len),
        })
    res = run_bass_kernel_spmd(nc, in_maps, core_ids=list(range(8)))

    out_pos = np.empty((nb, SUBSTEPS + 1, nv, 3), np.float32)
    out_vel = np.empty((nb, SUBSTEPS + 1, nv, 3), np.float32)
    for b in range(nb):
        r = res.results[b]
        out_pos[b] = unpermute_output(plan, r["opos"])
        out_vel[b] = unpermute_output(plan, r["ovel"])
    return out_pos, out_vel


# revision 7
# speedup vs baseline: 8.4108x; 2.4458x over previous
"""Mass-spring substep integrator on 8 Trainium2 NeuronCores.

Topology ("quad" edge-sharded, batch-replicated): every core carries all
4 batches; node ranks are dealt round-robin onto the 8 cores and each
core owns the incidences of its ranks, so the per-node force sums it
computes are complete — no cross-core force reduction. Once per substep
the integrated positions of each core's slab are AllGathered into a full
position table that next substep's gathers read.

Per-core data layout:
  - nodes are relabeled on the host: sorted by incidence count, grouped
    into "global ranks" of 128 (one node per SBUF partition), and global
    rank r = 8j + h is owned by core h with core-local rank j. State is
    node-major: pos[p, ((j*4 + b)*3 + c)] (m = 12 floats per node).
  - the owned directed incidences are laid out in a [128, J] slot grid
    grouped by owner rank, with a degree-profile template (segment size
    per rank j) SHARED across cores, so owner-side broadcast / segmented
    reduction are plain strided vector ops.
  - partner positions (48-byte per-node records covering all 4 batches)
    for a whole chunk of slot columns are fetched with ONE batched
    indirect DMA per chunk from the AllGathered table.

All index tables are precomputed on the host from the (static) edge
list; outputs are un-permuted back to the original node order on the
host after the device run.
"""

import numpy as np

import concourse.bass as bass
import concourse.mybir as mybir
import concourse.tile as tile
from concourse.bass_utils import run_bass_kernel_spmd

# Problem constants (must match the reference)
B, NV, NE, SUBSTEPS = 4, 100000, 400000, 10
DT = 0.01
K_SPRING = 1000.0
MASS = 1.0
DAMP = 0.999
ACT_SCALE = 0.1
EPS = 1e-6
GRAVITY_Y = -9.8

P = 128           # SBUF partitions
NCORE = 8
NCHUNK = 2        # slot-column chunks per substep
M = B * 3         # floats per node record
PAD_REST = float(np.sqrt(EPS))  # rest length that zeroes force on d=0 pad slots


# ---------------------------------------------------------------------------
# walrus workaround: this toolchain accepts only ONE sync-wait per
# instruction; split extra waits onto fresh same-engine NOPs.
# ---------------------------------------------------------------------------
_ctr = [0]


def _split_multi_waits(nc):
    for f in nc.m.functions:
        for b in f.blocks:
            old = b.instructions
            new = []
            changed = False
            for inst in old:
                si = inst.sync_info
                if si is not None and si.on_wait is not None and len(si.on_wait) > 1:
                    waits = list(si.on_wait)
                    for w in waits[:-1]:
                        _ctr[0] += 1
                        nop = mybir.InstNoOp(
                            name=f"SPLITW-{_ctr[0]}",
                            engine=inst.engine,
                            ins=[], outs=[],
                            sync_info=mybir.SyncInfo(on_wait=[w], on_update=[]),
                        )
                        new.append(nop)
                    si.on_wait = waits[-1:]
                    changed = True
                new.append(inst)
            if changed:
                b.instructions = new


class _TileContext(tile.TileContext):
    def __exit__(self, *args):
        r = super().__exit__(*args)
        if args[0] is None:
            _split_multi_waits(self.nc)
        return r


# ---------------------------------------------------------------------------
# Host-side plan construction (static, depends only on the edge list)
# ---------------------------------------------------------------------------
class Plan:
    pass


def build_plan(edges, nv, ne):
    """Relabel nodes by degree, deal ranks over cores, build the shared
    degree-profile slot template and per-core slot index tables."""
    nvp = -(-nv // P)
    nvp = -(-nvp // NCORE) * NCORE        # multiple of NCORE
    nvtot = nvp * P
    njc = nvp // NCORE                     # core-local ranks ("j")

    i_idx = edges[:, 0].astype(np.int64)
    j_idx = edges[:, 1].astype(np.int64)

    deg = np.bincount(edges.ravel(), minlength=nv).astype(np.int64)
    deg_pad = np.concatenate([deg, np.zeros(nvtot - nv, np.int64)])

    # global rank r <- the r-th group of 128 nodes in degree-sorted order
    order = np.argsort(-deg_pad, kind="stable")
    grid_nodes = order.reshape(nvp, P)          # [r, p]
    r_of = np.zeros(nvtot, np.int32)
    p_of = np.zeros(nvtot, np.int32)
    p_of[grid_nodes.ravel()] = np.tile(np.arange(P, dtype=np.int32), nvp)
    r_of[grid_nodes.ravel()] = np.repeat(np.arange(nvp, dtype=np.int32), P)

    dr = deg_pad[grid_nodes[:, 0]]              # max degree per global rank
    # shared per-core template: d_j = max over cores = dr[8j] (sorted desc)
    dj = dr[0::NCORE].copy()                    # [njc]
    seg_start = np.zeros(njc + 1, np.int64)
    seg_start[1:] = np.cumsum(dj)
    J = int(seg_start[-1])

    # degree classes: runs of equal dj with dj >= 1
    classes = []
    k = 0
    while k < njc:
        d = int(dj[k])
        k2 = k
        while k2 < njc and dj[k2] == d:
            k2 += 1
        if d >= 1:
            classes.append((k, k2, d))
        k = k2

    # chunks: split the slot range at rank boundaries
    targets = [J * (i + 1) // NCHUNK for i in range(NCHUNK)]
    bounds_k = [0]
    for t in targets[:-1]:
        kb = int(np.searchsorted(seg_start, t, side="left"))
        kb = max(min(kb, njc), bounds_k[-1])
        bounds_k.append(kb)
    bounds_k.append(njc)
    chunks = []
    for ci in range(NCHUNK):
        klo, khi = bounds_k[ci], bounds_k[ci + 1]
        lo, hi = int(seg_start[klo]), int(seg_start[khi])
        if hi == lo:
            continue
        pieces = []
        for (ka, kb, d) in classes:
            a, b2 = max(ka, klo), min(kb, khi)
            if a < b2:
                pieces.append((a, b2, d))
        chunks.append(dict(klo=klo, khi=khi, lo=lo, hi=hi, pieces=pieces))

    # gather-table row of a node: h = r % 8, j = r // 8 -> (h*128 + p)*njc + j
    h_of = (r_of % NCORE).astype(np.int64)
    j_of = (r_of // NCORE).astype(np.int64)
    row_of = (h_of * P + p_of) * njc + j_of     # [nvtot]

    # per-core slot tables
    iu = np.concatenate([i_idx, j_idx])
    iv = np.concatenate([j_idx, i_idx])
    ee = np.concatenate([np.arange(ne, dtype=np.int64)] * 2)
    hu = h_of[iu]
    ju = j_of[iu]
    pu = p_of[iu].astype(np.int64)

    rank_of_slot = np.repeat(np.arange(njc, dtype=np.int64), dj)  # [J]

    pidx = np.zeros((NCORE, P, J), np.int32)
    eidx = np.full((NCORE, P, J), -1, np.int64)
    for h in range(NCORE):
        sel = hu == h
        pv, jv, vv, ev = pu[sel], ju[sel], iv[sel], ee[sel]
        so = np.lexsort((ev, jv, pv))
        pv, jv, vv, ev = pv[so], jv[so], vv[so], ev[so]
        grp = pv * njc + jv
        uniq, first = np.unique(grp, return_index=True)
        within = np.arange(len(grp)) - np.repeat(
            first, np.diff(np.append(first, len(grp))))
        jpos = seg_start[jv] + within
        # pads point at the owner itself (d = 0)
        own_rows = ((h * P + np.arange(P, dtype=np.int64))[:, None] * njc
                    + rank_of_slot[None, :])
        pidx[h] = own_rows.astype(np.int32)
        pidx[h, pv, jpos] = row_of[vv].astype(np.int32)
        eidx[h, pv, jpos] = ev

    plan = Plan()
    plan.nv, plan.ne = nv, ne
    plan.nvp, plan.nvtot, plan.njc, plan.J = nvp, nvtot, njc, J
    plan.classes = classes
    plan.chunks = chunks
    plan.seg_start = seg_start
    plan.pidx = pidx
    plan.eidx = eidx
    plan.grid_nodes = grid_nodes
    plan.cw_max = max(c["hi"] - c["lo"] for c in chunks)
    return plan


def host_core_inputs(plan, h, input_pos, input_vel, input_action, rest_len):
    """Per-core initial state [P, njc*M], slot index table and -K*rest_eff."""
    njc, nv = plan.njc, plan.nv
    sub = plan.grid_nodes[h::NCORE]           # [njc, P] node ids
    valid = sub < nv
    gp = np.clip(sub, 0, nv - 1)
    ps = input_pos[:, gp].copy()              # [B, j, p, 3]
    vs = input_vel[:, gp].copy()
    ps[:, ~valid] = 0.0
    vs[:, ~valid] = 0.0
    # -> [p, j, b, c]
    pos = ps.transpose(2, 1, 0, 3).reshape(P, njc * M)
    vel = vs.transpose(2, 1, 0, 3).reshape(P, njc * M)

    e = plan.eidx[h]
    pad = e < 0
    ec = np.clip(e, 0, plan.ne - 1)
    rest_eff = rest_len[ec][None] * (
        1.0 + ACT_SCALE * np.tanh(input_action[:, ec]))   # [B, P, J]
    nkr = (-K_SPRING * rest_eff).astype(np.float32)
    nkr[:, pad] = -K_SPRING * PAD_REST
    nkr = np.ascontiguousarray(nkr.transpose(1, 2, 0).reshape(P, plan.J * B))
    return {
        "pos0": np.ascontiguousarray(pos, dtype=np.float32),
        "vel0": np.ascontiguousarray(vel, dtype=np.float32),
        "pidx": np.ascontiguousarray(plan.pidx[h]),
        "nkr": nkr,
    }


# ---------------------------------------------------------------------------
# Device kernel
# ---------------------------------------------------------------------------
def _ins_bcast(ap, pos_idx, count):
    dims = [list(x) for x in ap.ap]
    dims.insert(pos_idx, [0, count])
    return bass.AP(ap.tensor, ap.offset, dims)


def build_bass(plan, substeps):
    njc, J, nvtot = plan.njc, plan.J, plan.nvtot
    npm = njc * M
    cwm = plan.cw_max
    f32 = mybir.dt.float32

    nc = bass.Bass(num_devices=NCORE)
    pos0 = nc.dram_tensor("pos0", [P, npm], f32, kind="ExternalInput")
    vel0 = nc.dram_tensor("vel0", [P, npm], f32, kind="ExternalInput")
    pidx = nc.dram_tensor("pidx", [P, J], mybir.dt.int32, kind="ExternalInput")
    nkr_in = nc.dram_tensor("nkr", [P, J * B], f32, kind="ExternalInput")

    opos = nc.dram_tensor("opos", [substeps + 1, P, npm], f32,
                          kind="ExternalOutput")
    ovel = nc.dram_tensor("ovel", [substeps + 1, P, npm], f32,
                          kind="ExternalOutput")

    cc_in = nc.dram_tensor("cc_in", [P, npm], f32, kind="Internal")
    cc_out = nc.dram_tensor("cc_out", [NCORE, P, npm], f32, kind="Internal")
    # gather-table view: row (h*128+p)*njc + j holds that node's 12 floats
    tab = cc_out[:].rearrange("h p (j m) -> (h p j) m", m=M)

    with _TileContext(nc) as tc:
        with tc.tile_pool(name="state", bufs=1) as pool:
            pos = pool.tile([P, npm], f32, name="pos")
            vel = pool.tile([P, npm], f32, name="vel")
            fsum = pool.tile([P, npm], f32, name="fsum")
            pidx_sb = pool.tile([P, J], mybir.dt.int32, name="pidx_sb")
            nkr_sb = pool.tile([P, J * B], f32, name="nkr_sb")
            eps_t = pool.tile([P, 1], f32, name="eps_t")
            grav_t = pool.tile([P, 1], f32, name="grav_t")
            rem = [pool.tile([P, cwm * M], f32, name=f"rem{b}")
                   for b in range(2)]
            sq = [pool.tile([P, cwm * M], f32, name=f"sq{b}")
                  for b in range(2)]
            s2 = [pool.tile([P, cwm * B], f32, name=f"s2{b}")
                  for b in range(2)]
            inv = [pool.tile([P, cwm * B], f32, name=f"inv{b}")
                   for b in range(2)]

            pos_km = pos[:].rearrange("p (k m) -> p k m", m=M)
            fsum_km = fsum[:].rearrange("p (k m) -> p k m", m=M)

            # ---- one-time setup ----
            nc.vector.memset(eps_t[:], float(EPS))
            nc.vector.memset(grav_t[:], float(GRAVITY_Y * DT))
            nc.vector.memset(fsum[:], 0.0)
            nc.sync.dma_start(pos[:], pos0[:])
            nc.sync.dma_start(vel[:], vel0[:])
            nc.sync.dma_start(pidx_sb[:], pidx[:])
            nc.sync.dma_start(nkr_sb[:], nkr_in[:])
            nc.sync.dma_start(opos[0], pos[:])
            nc.sync.dma_start(ovel[0], vel[:])

            # ---- substeps (statically unrolled) ----
            for s in range(substeps):
                # 1) share this core's slab; AllGather the full table
                nc.sync.dma_start(cc_in[:], pos[:])
                nc.gpsimd.collective_compute(
                    "AllGather", mybir.AluOpType.bypass,
                    replica_groups=[list(range(NCORE))],
                    ins=[cc_in[:]], outs=[cc_out[:]],
                )

                for ci, ch in enumerate(plan.chunks):
                    bi = ci % 2
                    lo, hi = ch["lo"], ch["hi"]
                    cw = hi - lo
                    remc = rem[bi][:, :cw * M]
                    sqc = sq[bi][:, :cw * M]
                    s2c = s2[bi][:, :cw * B]
                    invc = inv[bi][:, :cw * B]
                    rem_jm = remc.rearrange("p (j m) -> p j m", m=M)
                    rem_jbc = remc.rearrange("p (j b c) -> p j b c", b=B, c=3)

                    # 2) gather partner records (one indirect DMA)
                    nc.gpsimd.indirect_dma_start(
                        out=remc,
                        out_offset=None,
                        in_=tab,
                        in_offset=bass.IndirectOffsetOnAxis(
                            ap=pidx_sb[:, lo:hi], axis=0),
                    )
                    # 3) d = partner - owner (per degree-class piece)
                    for (ka, kb, d) in ch["pieces"]:
                        s0 = int(plan.seg_start[ka]) - lo
                        nk = kb - ka
                        dst = rem_jm[:, s0:s0 + nk * d, :].rearrange(
                            "p (n dd) m -> p n dd m", dd=d)
                        src = _ins_bcast(pos_km[:, ka:kb, :], 2, d)
                        nc.vector.tensor_tensor(
                            out=dst, in0=dst, in1=src,
                            op=mybir.AluOpType.subtract)
                    # 4) sq = d*d (ACT), s2 = sum_c sq (DVE)
                    nc.scalar.activation(
                        sqc, remc, mybir.ActivationFunctionType.Square)
                    nc.vector.tensor_reduce(
                        out=s2c.rearrange("p (j b) -> p j b", b=B),
                        in_=sqc.rearrange("p (j b c) -> p j b c", b=B, c=3),
                        axis=mybir.AxisListType.X, op=mybir.AluOpType.add)
                    # 5) len = sqrt(s2 + eps) (ACT, in place)
                    nc.scalar.activation(
                        s2c, s2c, mybir.ActivationFunctionType.Sqrt,
                        bias=eps_t[:])
                    # 6) inv = 1/len ; t = nkr*inv  (in place in inv)
                    nc.vector.reciprocal(invc, s2c)
                    nc.vector.tensor_tensor(
                        out=invc, in0=nkr_sb[:, lo * B:hi * B], in1=invc,
                        op=mybir.AluOpType.mult)
                    # 7) f = (t + K) * d   (in place on rem)
                    tb = _ins_bcast(
                        invc.rearrange("p (j b) -> p j b", b=B), 3, 3)
                    nc.vector.scalar_tensor_tensor(
                        out=rem_jbc, in0=tb, scalar=float(K_SPRING),
                        in1=rem_jbc, op0=mybir.AluOpType.add,
                        op1=mybir.AluOpType.mult)
                    # 8) segmented reduce -> fsum
                    for (ka, kb, d) in ch["pieces"]:
                        s0 = int(plan.seg_start[ka]) - lo
                        nk = kb - ka
                        src = rem_jm[:, s0:s0 + nk * d, :].rearrange(
                            "p (n dd) m -> p n m dd", dd=d)
                        dst = fsum_km[:, ka:kb, :]
                        nc.vector.tensor_reduce(
                            out=dst, in_=src, axis=mybir.AxisListType.X,
                            op=mybir.AluOpType.add)

                # 9) integrate:
                #    t = fsum*DT + vel ; t_y += DT*G ; vel = DAMP*t ;
                #    pos = vel*DT + pos
                nc.vector.scalar_tensor_tensor(
                    out=fsum[:], in0=fsum[:], scalar=float(DT / MASS),
                    in1=vel[:], op0=mybir.AluOpType.mult,
                    op1=mybir.AluOpType.add)
                yv = fsum[:].rearrange("p (q c) -> p q c", c=3)[:, :, 1:2]
                nc.scalar.activation(
                    yv, yv, mybir.ActivationFunctionType.Identity,
                    bias=grav_t[:])
                nc.scalar.activation(
                    vel[:], fsum[:], mybir.ActivationFunctionType.Copy,
                    scale=float(DAMP))
                nc.vector.scalar_tensor_tensor(
                    out=pos[:], in0=vel[:], scalar=float(DT),
                    in1=pos[:], op0=mybir.AluOpType.mult,
                    op1=mybir.AluOpType.add)
                # 10) trajectory
                nc.sync.dma_start(opos[s + 1], pos[:])
                nc.sync.dma_start(ovel[s + 1], vel[:])

    return nc


# ---------------------------------------------------------------------------
# Entry point
# ---------------------------------------------------------------------------
_cache = {}


def _get_plan_and_bass(edges, nv, ne, substeps):
    kh = (hash(edges.tobytes()), nv, ne, substeps)
    if kh not in _cache:
        plan = build_plan(edges, nv, ne)
        nc = build_bass(plan, substeps)
        _cache[kh] = (plan, nc)
    return _cache[kh]


def kernel(input_action, input_pos, input_vel, rest_len, edges):
    input_action = np.asarray(input_action, np.float32)
    input_pos = np.asarray(input_pos, np.float32)
    input_vel = np.asarray(input_vel, np.float32)
    rest_len = np.asarray(rest_len, np.float32)
    edges = np.asarray(edges, np.int32)

    nb, nv, _ = input_pos.shape
    ne = edges.shape[0]
    plan, nc = _get_plan_and_bass(edges, nv, ne, SUBSTEPS)

    in_maps = [
        host_core_inputs(plan, h, input_pos, input_vel, input_action, rest_len)
        for h in range(NCORE)
    ]
    res = run_bass_kernel_spmd(nc, in_maps, core_ids=list(range(NCORE)))

    s1 = SUBSTEPS + 1
    out_pos = np.empty((nb, s1, nv, 3), np.float32)
    out_vel = np.empty((nb, s1, nv, 3), np.float32)
    for h in range(NCORE):
        sub = plan.grid_nodes[h::NCORE]       # [njc, P]
        mask = sub < nv
        jj, pp = np.nonzero(mask)
        ids = sub[jj, pp]
        r = res.results[h]
        tp = r["opos"].reshape(s1, P, plan.njc, nb, 3)
        tv = r["ovel"].reshape(s1, P, plan.njc, nb, 3)
        for b in range(nb):
            out_pos[b][:, ids] = tp[:, pp, jj, b]
            out_vel[b][:, ids] = tv[:, pp, jj, b]
    return out_pos, out_vel
